# revision 6
# baseline (speedup 1.0000x reference)
"""BatchedFoveator Trainium2 kernel.

The reference computes an integral image (double cumsum) and gathers 4
corners per output pixel.  Mathematically that is exactly multi-scale
average pooling of the input image:

  level 0 (stride 1, 64 tokens): center crop  rows/cols [192, 320)
  level 1 (stride 2, 48 tokens): 2x2 average pool of  [128, 384)^2 (ring)
  level 2 (stride 4, 48 tokens): 4x4 average pool of full image   (ring)

Each level yields a 128x128 map per (b, c); token (gy, gx) of a level is
the 16x16 tile map[16gy:16gy+16, 16gx:16gx+16].  Ring order for levels
1/2: rows gy=0,1 (all gx), then gy=2..5 with gx in {0,1,6,7}, then
gy=6,7 (all gx).

Sharding: pure data parallel, batch 32 -> 4 images per core x 8 cores.

Per-core pipeline (per image b of its 4):
  1. SWDGE accum loads: even rows -> P1v, odd rows accumulated (CCE add)
     => P1v holds vertical 2-row sums, partition p has pooled rows 2p,2p+1.
  2. HWDGE load of center crop -> C [p=row-192][c*128+x].
  3. DVE h-add (stride 2)      -> P1u  (2x2 box sums).
  4. ACT scale crop of P1u * 0.25 -> M1pre; SBUF->SBUF DMA rearrange to
     M1 [p=row][c*128+X]  (level-1 map, one row per partition).
  5. DVE v-add + h-add of P1u -> 4x4 sums; ACT * 1/16 -> M4 [p=row][c*128+X].
  6. SBUF->SBUF gather DMAs assemble U [p = b*32+nh*16+i][c*1280+n'*16+j].
  7. HWDGE out DMAs: per (b, c, half): U -> out[b, 80nh:80nh+80, c].
"""

import os
import threading

import numpy as np

N_CORES = 8
B_FULL = 32
B_SHARD = B_FULL // N_CORES  # 4
C = 3
S = 512
T = 16

_lock = threading.Lock()
_cache = {}


def _ring_chunks():
    """Chunks of ring tokens for levels 1/2: (k0, gy, x0, width_tokens).

    k0 = index of first token within the level's 48; x0 = first map column
    (in elements); width_tokens = tokens covered (contiguous in both n and
    map columns)."""
    chunks = []
    k = 0
    for gy in (0, 1):
        chunks.append((k, gy, 0, 8))
        k += 8
    for gy in (2, 3, 4, 5):
        chunks.append((k, gy, 0, 2))       # gx 0,1
        chunks.append((k + 2, gy, 96, 2))  # gx 6,7
        k += 4
    for gy in (6, 7):
        chunks.append((k, gy, 0, 8))
        k += 8
    return chunks


def _build_module():
    import concourse.bacc as bacc
    import concourse.mybir as mybir
    import concourse.tile as tile

    nc = bacc.Bacc("TRN2", target_bir_lowering=False, debug=False)
    f32 = mybir.dt.float32

    images = nc.dram_tensor("images", (B_SHARD, C, S, S), f32, kind="ExternalInput")
    out = nc.dram_tensor("out", (B_SHARD, 160, C, T, T), f32, kind="ExternalOutput")

    img = images.ap()
    outp = out.ap()

    with tile.TileContext(nc) as tc:
        with (
            tc.tile_pool(name="p1v", bufs=2) as pool_p1v,
            tc.tile_pool(name="crop", bufs=2) as pool_crop,
            tc.tile_pool(name="p1u", bufs=2) as pool_p1u,
            tc.tile_pool(name="m1pre", bufs=2) as pool_m1pre,
            tc.tile_pool(name="m1", bufs=2) as pool_m1,
            tc.tile_pool(name="v4", bufs=2) as pool_v4,
            tc.tile_pool(name="m4t", bufs=2) as pool_m4t,
            tc.tile_pool(name="m4", bufs=2) as pool_m4,
            tc.tile_pool(name="uout", bufs=1) as pool_u,
        ):
            # U holds the fully assembled output for all 4 images:
            # partition p = b*32 + nh*16 + i, free = c*1280 + n'*16 + j
            # (nh = n // 80, n' = n % 80).
            U = pool_u.tile([128, 3 * 80 * T], f32, name="U")

            for b in range(B_SHARD):
                # ---- 1. vertical-pair pooling during load (CCE add) ----
                # P1v free layout: c*1024 + r2*512 + x ; partition p holds
                # pooled rows 2p (r2=0) and 2p+1 (r2=1).
                P1v = pool_p1v.tile([128, C * 1024], f32, name="P1v")
                p1v_v = P1v.rearrange("p (c r2 x) -> p c r2 x", c=C, r2=2)
                # image row = 4p + 2*r2 + e
                img_b = img[b].rearrange("c (p r2 e) x -> p c r2 e x", r2=2, e=2)
                for c in range(C):
                    nc.gpsimd.dma_start(
                        out=p1v_v[:, c], in_=img_b[:, c, :, 0]
                    )
                    nc.gpsimd.dma_start(
                        out=p1v_v[:, c],
                        in_=img_b[:, c, :, 1],
                        accum_op=mybir.AluOpType.add,
                    )

                # ---- 2. center crop load: Ct [p=row-192][c*128+x] ----
                Ct = pool_crop.tile([128, C * 128], f32, name="Ct")
                nc.sync.dma_start(
                    out=Ct[:],
                    in_=img[b, :, 192:320, 192:320].transpose([1, 0, 2]),
                )

                # ---- 3. horizontal pairs: P1u = 2x2 box sums ----
                # P1u free: cr*256 + xp  (cr = 2c + r2)
                P1u = pool_p1u.tile([128, C * 512], f32, name="P1u")
                p1v_h = P1v.rearrange("p (cr xp par) -> p cr xp par", cr=2 * C, par=2)
                p1u_v = P1u.rearrange("p (cr xp) -> p cr xp", cr=2 * C)
                nc.vector.tensor_add(
                    out=p1u_v, in0=p1v_h[:, :, :, 0], in1=p1v_h[:, :, :, 1]
                )

                # ---- 4. level-1 map: scale crop, rearrange to row/partition ----
                # M1pre free: cr*128 + X  (only partitions 32..96 meaningful)
                M1pre = pool_m1pre.tile([128, C * 256], f32, name="M1pre")
                m1pre_v = M1pre.rearrange("p (cr x) -> p cr x", cr=2 * C)
                # engine APs with base partition 32 may span at most 32
                # partitions -> two 32-partition ops
                nc.scalar.mul(m1pre_v[32:64], p1u_v[32:64, :, 64:192], 0.25)
                nc.scalar.mul(m1pre_v[64:96], p1u_v[64:96, :, 64:192], 0.25)
                # Rearrange to one map-row per partition.  NOTE: a
                # partition-split *dst* AP ([r1 st2][r2 st1][x]) passes the
                # BIR verifier but writes garbage on odd partitions, so the
                # row-pair split must live on the *src* free side instead:
                # src [64 parts][r2:2][x] -> dst [128 parts][x].
                M1 = pool_m1.tile([128, C * 128], f32, name="M1")
                m1_dst = M1.rearrange("p (c x) -> p c x", c=C)
                m1pre_src = M1pre.rearrange("p (c r2 x) -> p c r2 x", c=C, r2=2)
                for c in range(C):
                    nc.sync.dma_start(
                        out=m1_dst[:, c], in_=m1pre_src[32:96, c]
                    )

                # ---- 5. level-2 map: pool P1u 2x2 again, scale 1/16 ----
                V4 = pool_v4.tile([128, C * 256], f32, name="V4")
                p1u_c = P1u.rearrange("p (c r2 xp) -> p c r2 xp", c=C, r2=2)
                v4_v = V4.rearrange("p (c xp) -> p c xp", c=C)
                nc.vector.tensor_add(
                    out=v4_v, in0=p1u_c[:, :, 0], in1=p1u_c[:, :, 1]
                )
                M4t = pool_m4t.tile([128, C * 128], f32, name="M4t")
                v4_h = V4.rearrange("p (c X par) -> p c X par", c=C, par=2)
                m4t_v = M4t.rearrange("p (c X) -> p c X", c=C)
                nc.vector.tensor_add(
                    out=m4t_v, in0=v4_h[:, :, :, 0], in1=v4_h[:, :, :, 1]
                )
                M4 = pool_m4.tile([128, C * 128], f32, name="M4")
                nc.scalar.mul(M4[:], M4t[:], 1.0 / 16.0)

                # ---- 6. gather token tiles into U ----
                def gather(map_tile, n0, k0, gy, x0, wtok):
                    """Copy tokens [n0+k0, n0+k0+wtok) (= map row-block gy,
                    cols [x0, x0+16*wtok)) of map_tile into U.  Both sides
                    keep the partition dim (i) outermost."""
                    n = n0 + k0
                    nh, np_ = divmod(n, 80)
                    pbase = b * 32 + nh * 16
                    src = map_tile[16 * gy : 16 * gy + 16].rearrange(
                        "i (c x) -> i c x", c=C
                    )[:, :, x0 : x0 + 16 * wtok]
                    dst = U[pbase : pbase + 16].rearrange(
                        "i (c f) -> i c f", c=C
                    )[:, :, np_ * 16 : np_ * 16 + 16 * wtok]
                    nc.sync.dma_start(out=dst, in_=src)

                # level 0: full 8x8 grid, n = 8*gy + gx
                for gy in range(8):
                    gather(Ct, 0, 8 * gy, gy, 0, 8)
                # levels 1, 2: ring order
                for n0, m in ((64, M1), (112, M4)):
                    for k0, gy, x0, wtok in _ring_chunks():
                        gather(m, n0, k0, gy, x0, wtok)

                # ---- 7. output DMAs: per (c, half) ----
                # SBUF side iterates (i, n', j); DRAM side matches with
                # dims [i (st 16)][n' (st 768)][j (st 1)] via transpose.
                for nh in range(2):
                    pbase = b * 32 + nh * 16
                    u_src = U[pbase : pbase + 16].rearrange(
                        "i (c n j) -> i c n j", c=C, n=80
                    )
                    for c in range(C):
                        nc.scalar.dma_start(
                            out=outp[b, 80 * nh : 80 * nh + 80, c].transpose(
                                [1, 0, 2]
                            ),
                            in_=u_src[:, c],
                        )

    nc.compile()
    return nc


def _get_module():
    with _lock:
        if "nc" not in _cache:
            _cache["nc"] = _build_module()
        return _cache["nc"]


def kernel(images: np.ndarray) -> np.ndarray:
    from concourse.bass_utils import run_bass_kernel_spmd

    images = np.ascontiguousarray(np.asarray(images, dtype=np.float32))
    assert images.shape == (B_FULL, C, S, S), images.shape

    nc = _get_module()
    in_maps = [
        {"images": images[k * B_SHARD : (k + 1) * B_SHARD]} for k in range(N_CORES)
    ]
    res = run_bass_kernel_spmd(
        nc,
        in_maps,
        core_ids=list(range(N_CORES)),
        trace=bool(int(os.environ.get("FOV_TRACE", "0"))),
    )
    _cache["last_results"] = res
    out = np.concatenate([r["out"] for r in res.results], axis=0)
    return out


if __name__ == "__main__":
    x = np.random.randn(B_FULL, C, S, S).astype(np.float32)
    y = kernel(x)
    print("out", y.shape, y.dtype, float(np.abs(y).max()))


# revision 9
# speedup vs baseline: 1.0219x; 1.0219x over previous
"""BatchedFoveator Trainium2 kernel.

The reference computes an integral image (double cumsum) and gathers 4
corners per output pixel.  Mathematically that is exactly multi-scale
average pooling of the input image:

  level 0 (stride 1, 64 tokens): center crop  rows/cols [192, 320)
  level 1 (stride 2, 48 tokens): 2x2 average pool of  [128, 384)^2 (ring)
  level 2 (stride 4, 48 tokens): 4x4 average pool of full image   (ring)

Each level yields a 128x128 map per (b, c); token (gy, gx) of a level is
the 16x16 tile map[16gy:16gy+16, 16gx:16gx+16].  Ring order for levels
1/2: rows gy=0,1 (all gx), then gy=2..5 with gx in {0,1,6,7}, then
gy=6,7 (all gx).

Sharding: pure data parallel, batch 32 -> 4 images per core x 8 cores.

Per-core pipeline (per image b of its 4):
  1. SWDGE accum loads: even rows -> P1v, odd rows accumulated (CCE add)
     => P1v holds vertical 2-row sums, partition p has pooled rows 2p,2p+1.
  2. HWDGE load of center crop -> C [p=row-192][c*128+x].
  3. DVE h-add (stride 2)      -> P1u  (2x2 box sums).
  4. ACT scale crop of P1u * 0.25 -> M1pre; SBUF->SBUF DMA rearrange to
     M1 [p=row][c*128+X]  (level-1 map, one row per partition).
  5. DVE v-add + h-add of P1u -> 4x4 sums; ACT * 1/16 -> M4 [p=row][c*128+X].
  6. SBUF->SBUF gather DMAs assemble U [p = b*32+nh*16+i][c*1280+n'*16+j].
  7. HWDGE out DMAs: per (b, c, half): U -> out[b, 80nh:80nh+80, c].
"""

import os
import threading

import numpy as np

N_CORES = 8
B_FULL = 32
B_SHARD = B_FULL // N_CORES  # 4
C = 3
S = 512
T = 16

_lock = threading.Lock()
_cache = {}


def _ring_chunks():
    """Chunks of ring tokens for levels 1/2: (k0, gy, x0, width_tokens).

    k0 = index of first token within the level's 48; x0 = first map column
    (in elements); width_tokens = tokens covered (contiguous in both n and
    map columns)."""
    chunks = []
    k = 0
    for gy in (0, 1):
        chunks.append((k, gy, 0, 8))
        k += 8
    for gy in (2, 3, 4, 5):
        chunks.append((k, gy, 0, 2))       # gx 0,1
        chunks.append((k + 2, gy, 96, 2))  # gx 6,7
        k += 4
    for gy in (6, 7):
        chunks.append((k, gy, 0, 8))
        k += 8
    return chunks


def _build_module():
    import concourse.bacc as bacc
    import concourse.mybir as mybir
    import concourse.tile as tile

    nc = bacc.Bacc("TRN2", target_bir_lowering=False, debug=False)
    f32 = mybir.dt.float32

    images = nc.dram_tensor("images", (B_SHARD, C, S, S), f32, kind="ExternalInput")
    out = nc.dram_tensor("out", (B_SHARD, 160, C, T, T), f32, kind="ExternalOutput")

    img = images.ap()
    outp = out.ap()

    with tile.TileContext(nc) as tc:
        with (
            tc.tile_pool(name="img", bufs=2) as pool_img,
            tc.tile_pool(name="p1v", bufs=2) as pool_p1v,
            tc.tile_pool(name="crop", bufs=2) as pool_crop,
            tc.tile_pool(name="p1u", bufs=2) as pool_p1u,
            tc.tile_pool(name="m1pre", bufs=2) as pool_m1pre,
            tc.tile_pool(name="m1", bufs=2) as pool_m1,
            tc.tile_pool(name="v4", bufs=2) as pool_v4,
            tc.tile_pool(name="m4t", bufs=2) as pool_m4t,
            tc.tile_pool(name="m4", bufs=2) as pool_m4,
            tc.tile_pool(name="uout", bufs=1) as pool_u,
        ):
            # U holds the fully assembled output for all 4 images:
            # partition p = b*32 + nh*16 + i, free = c*1280 + n'*16 + j
            # (nh = n // 80, n' = n % 80).
            U = pool_u.tile([128, 3 * 80 * T], f32, name="U")

            for b in range(B_SHARD):
                # ---- 1. load full image plane-interleaved + v-pair add ----
                # It free layout: c*2048 + r*512 + x  (partition p = rows
                # 4p..4p+4); DRAM runs are 8KB -> near line rate.
                It = pool_img.tile([128, C * 2048], f32, name="It")
                img_b = img[b].rearrange("c (p r) x -> p c (r x)", p=128)
                nc.sync.dma_start(out=It[:], in_=img_b)
                # P1v free layout: cr*512 + x  (cr = 2c + r2); partition p
                # holds pooled rows 2p (r2=0) and 2p+1 (r2=1).
                # It free = c*2048 + r2*1024 + e*512 + x = cr*1024 + e*512 + x.
                P1v = pool_p1v.tile([128, C * 1024], f32, name="P1v")
                it_v = It.rearrange("p (cr e x) -> p cr e x", cr=2 * C, e=2)
                p1v_v3 = P1v.rearrange("p (cr x) -> p cr x", cr=2 * C)
                nc.vector.tensor_add(
                    out=p1v_v3, in0=it_v[:, :, 0], in1=it_v[:, :, 1]
                )

                # ---- 2. center crop load: Ct [p=row-192][c*128+x] ----
                Ct = pool_crop.tile([128, C * 128], f32, name="Ct")
                nc.sync.dma_start(
                    out=Ct[:],
                    in_=img[b, :, 192:320, 192:320].transpose([1, 0, 2]),
                )

                # ---- 3. horizontal pairs: P1u = 2x2 box sums ----
                # P1u free: cr*256 + xp  (cr = 2c + r2)
                P1u = pool_p1u.tile([128, C * 512], f32, name="P1u")
                p1v_h = P1v.rearrange("p (cr xp par) -> p cr xp par", cr=2 * C, par=2)
                p1u_v = P1u.rearrange("p (cr xp) -> p cr xp", cr=2 * C)
                nc.vector.tensor_add(
                    out=p1u_v, in0=p1v_h[:, :, :, 0], in1=p1v_h[:, :, :, 1]
                )

                # ---- 4. level-1 map: scale crop, rearrange to row/partition ----
                # M1pre free: cr*128 + X  (only partitions 32..96 meaningful)
                M1pre = pool_m1pre.tile([128, C * 256], f32, name="M1pre")
                m1pre_v = M1pre.rearrange("p (cr x) -> p cr x", cr=2 * C)
                # engine APs with base partition 32 may span at most 32
                # partitions -> two 32-partition ops
                nc.scalar.mul(m1pre_v[32:64], p1u_v[32:64, :, 64:192], 0.25)
                nc.scalar.mul(m1pre_v[64:96], p1u_v[64:96, :, 64:192], 0.25)
                # Rearrange to one map-row per partition.  NOTE: a
                # partition-split *dst* AP ([r1 st2][r2 st1][x]) passes the
                # BIR verifier but writes garbage on odd partitions, so the
                # row-pair split must live on the *src* free side instead:
                # src [64 parts][r2:2][x] -> dst [128 parts][x].
                M1 = pool_m1.tile([128, C * 128], f32, name="M1")
                m1_dst = M1.rearrange("p (c x) -> p c x", c=C)
                m1pre_src = M1pre.rearrange("p (c r2 x) -> p c r2 x", c=C, r2=2)
                for c in range(C):
                    nc.sync.dma_start(
                        out=m1_dst[:, c], in_=m1pre_src[32:96, c]
                    )

                # ---- 5. level-2 map: pool P1u 2x2 again, scale 1/16 ----
                V4 = pool_v4.tile([128, C * 256], f32, name="V4")
                p1u_c = P1u.rearrange("p (c r2 xp) -> p c r2 xp", c=C, r2=2)
                v4_v = V4.rearrange("p (c xp) -> p c xp", c=C)
                nc.vector.tensor_add(
                    out=v4_v, in0=p1u_c[:, :, 0], in1=p1u_c[:, :, 1]
                )
                M4t = pool_m4t.tile([128, C * 128], f32, name="M4t")
                v4_h = V4.rearrange("p (c X par) -> p c X par", c=C, par=2)
                m4t_v = M4t.rearrange("p (c X) -> p c X", c=C)
                nc.vector.tensor_add(
                    out=m4t_v, in0=v4_h[:, :, :, 0], in1=v4_h[:, :, :, 1]
                )
                M4 = pool_m4.tile([128, C * 128], f32, name="M4")
                nc.scalar.mul(M4[:], M4t[:], 1.0 / 16.0)

                # ---- 6. gather token tiles into U ----
                def gather(map_tile, n0, k0, gy, x0, wtok):
                    """Copy tokens [n0+k0, n0+k0+wtok) (= map row-block gy,
                    cols [x0, x0+16*wtok)) of map_tile into U.  Both sides
                    keep the partition dim (i) outermost."""
                    n = n0 + k0
                    nh, np_ = divmod(n, 80)
                    pbase = b * 32 + nh * 16
                    src = map_tile[16 * gy : 16 * gy + 16].rearrange(
                        "i (c x) -> i c x", c=C
                    )[:, :, x0 : x0 + 16 * wtok]
                    dst = U[pbase : pbase + 16].rearrange(
                        "i (c f) -> i c f", c=C
                    )[:, :, np_ * 16 : np_ * 16 + 16 * wtok]
                    nc.sync.dma_start(out=dst, in_=src)

                # level 0: full 8x8 grid, n = 8*gy + gx
                for gy in range(8):
                    gather(Ct, 0, 8 * gy, gy, 0, 8)
                # levels 1, 2: ring order
                for n0, m in ((64, M1), (112, M4)):
                    for k0, gy, x0, wtok in _ring_chunks():
                        gather(m, n0, k0, gy, x0, wtok)

                # ---- 7. output DMAs: per (c, half) ----
                # SBUF side iterates (i, n', j); DRAM side matches with
                # dims [i (st 16)][n' (st 768)][j (st 1)] via transpose.
                for nh in range(2):
                    pbase = b * 32 + nh * 16
                    u_src = U[pbase : pbase + 16].rearrange(
                        "i (c n j) -> i c n j", c=C, n=80
                    )
                    for c in range(C):
                        nc.scalar.dma_start(
                            out=outp[b, 80 * nh : 80 * nh + 80, c].transpose(
                                [1, 0, 2]
                            ),
                            in_=u_src[:, c],
                        )

    nc.compile()
    return nc


def _get_module():
    with _lock:
        if "nc" not in _cache:
            _cache["nc"] = _build_module()
        return _cache["nc"]


def kernel(images: np.ndarray) -> np.ndarray:
    from concourse.bass_utils import run_bass_kernel_spmd

    images = np.ascontiguousarray(np.asarray(images, dtype=np.float32))
    assert images.shape == (B_FULL, C, S, S), images.shape

    nc = _get_module()
    in_maps = [
        {"images": images[k * B_SHARD : (k + 1) * B_SHARD]} for k in range(N_CORES)
    ]
    res = run_bass_kernel_spmd(
        nc,
        in_maps,
        core_ids=list(range(N_CORES)),
        trace=bool(int(os.environ.get("FOV_TRACE", "0"))),
    )
    _cache["last_results"] = res
    out = np.concatenate([r["out"] for r in res.results], axis=0)
    return out


if __name__ == "__main__":
    x = np.random.randn(B_FULL, C, S, S).astype(np.float32)
    y = kernel(x)
    print("out", y.shape, y.dtype, float(np.abs(y).max()))


# revision 10
# speedup vs baseline: 1.2648x; 1.2377x over previous
"""BatchedFoveator Trainium2 kernel.

The reference computes an integral image (double cumsum) and gathers 4
corners per output pixel.  Mathematically that is exactly multi-scale
average pooling of the input image:

  level 0 (stride 1, 64 tokens): center crop  rows/cols [192, 320)
  level 1 (stride 2, 48 tokens): 2x2 average pool of  [128, 384)^2 (ring)
  level 2 (stride 4, 48 tokens): 4x4 average pool of full image   (ring)

Each level yields a 128x128 map per (b, c); token (gy, gx) of a level is
the 16x16 tile map[16gy:16gy+16, 16gx:16gx+16].  Ring order for levels
1/2: rows gy=0,1 (all gx), then gy=2..5 with gx in {0,1,6,7}, then
gy=6,7 (all gx).

Sharding: pure data parallel, batch 32 -> 4 images per core x 8 cores.

Per-core pipeline (per image b of its 4):
  1. One 3MB HWDGE load -> It [p = rows 4p..4p+4][c*2048 + r*512 + x].
  2. DVE adds: vertical pairs -> P1v, horizontal pairs -> P1u (2x2 box
     sums, two pooled rows per partition).
  3. Level-1 map: ACT-scale center of P1u -> M1pre (r2-major), one
     SBUF->SBUF DMA -> M1 [p = map row][c*128 + X]; ACT compacts the
     ring-middle columns -> M1m.
  4. Level-2 map: DVE pool P1u again -> M4t, ACT scale -> M4 + M4m.
  5. Gather DMAs (SBUF->SBUF) assemble U [p = b*32 + nh*16 + i]
     [c*1280 + n'*16 + j]  (nh = n//80, n' = n%80); level-0 tokens are
     DMA'd DRAM->U straight from the input image crop.
  6. Out DMAs per (b, nh, c): U -> out[b, 80nh:80nh+80, c].

DMA instruction count is the main cost driver (HWDGE issue is ~0.6us
per dma_start), so transfers are batched to the 3-dim AP limit and
spread across the sync/scalar HWDGE rings and the gpsimd SWDGE path.
"""

import os
import threading

import numpy as np

N_CORES = 8
B_FULL = 32
B_SHARD = B_FULL // N_CORES  # 4
C = 3
S = 512
T = 16

_lock = threading.Lock()
_cache = {}


def _build_module():
    import concourse.bacc as bacc
    import concourse.mybir as mybir
    import concourse.tile as tile

    nc = bacc.Bacc("TRN2", target_bir_lowering=False, debug=False)
    f32 = mybir.dt.float32

    images = nc.dram_tensor("images", (B_SHARD, C, S, S), f32, kind="ExternalInput")
    out = nc.dram_tensor("out", (B_SHARD, 160, C, T, T), f32, kind="ExternalOutput")

    img = images.ap()
    outp = out.ap()

    with tile.TileContext(nc) as tc:
        with (
            tc.tile_pool(name="img", bufs=2) as pool_img,
            tc.tile_pool(name="p1v", bufs=2) as pool_p1v,
            tc.tile_pool(name="p1u", bufs=2) as pool_p1u,
            tc.tile_pool(name="m1pre", bufs=2) as pool_m1pre,
            tc.tile_pool(name="m1", bufs=2) as pool_m1,
            tc.tile_pool(name="m1m", bufs=2) as pool_m1m,
            tc.tile_pool(name="v4", bufs=2) as pool_v4,
            tc.tile_pool(name="m4t", bufs=2) as pool_m4t,
            tc.tile_pool(name="m4", bufs=2) as pool_m4,
            tc.tile_pool(name="m4m", bufs=2) as pool_m4m,
            tc.tile_pool(name="uout", bufs=1) as pool_u,
        ):
            # U holds the fully assembled output for all 4 images:
            # partition p = b*32 + nh*16 + i, free = c*1280 + n'*16 + j.
            U = pool_u.tile([128, C * 80 * T], f32, name="U")

            for b in range(B_SHARD):
                # ---- 1. full image load ----
                # It free: c*2048 + r*512 + x  (partition p = rows 4p..4p+4,
                # r = row%4 = 2*r2 + e); 8KB DRAM runs.
                It = pool_img.tile([128, C * 2048], f32, name="It")
                img_b = img[b].rearrange("c (p r) x -> p c (r x)", p=128)
                nc.sync.dma_start(out=It[:], in_=img_b)

                # ---- 2. 2x2 box sums ----
                # P1v free: cr*512 + x  (cr = 2c + r2); partition p holds
                # pooled rows 2p (r2=0), 2p+1 (r2=1).
                P1v = pool_p1v.tile([128, C * 1024], f32, name="P1v")
                it_v = It.rearrange("p (cr e x) -> p cr e x", cr=2 * C, e=2)
                p1v_v3 = P1v.rearrange("p (cr x) -> p cr x", cr=2 * C)
                nc.vector.tensor_add(
                    out=p1v_v3, in0=it_v[:, :, 0], in1=it_v[:, :, 1]
                )
                # P1u free: cr*256 + xp
                P1u = pool_p1u.tile([128, C * 512], f32, name="P1u")
                p1v_h = P1v.rearrange("p (cr xp par) -> p cr xp par", cr=2 * C, par=2)
                p1u_v = P1u.rearrange("p (cr xp) -> p cr xp", cr=2 * C)
                nc.vector.tensor_add(
                    out=p1u_v, in0=p1v_h[:, :, :, 0], in1=p1v_h[:, :, :, 1]
                )

                # ---- 3. level-1 map ----
                # M1pre free: r2*384 + c*128 + X (r2-major so the rearrange
                # to M1 is a single DMA); partitions 32..96.
                # Engine APs starting at partition 32 may span <=32
                # partitions -> split ops per partition half.
                # P1u crop: cols xp 64..192 of cr = 2c+r2.
                M1pre = pool_m1pre.tile([128, 2 * C * 128], f32, name="M1pre")
                p1u_c4 = P1u.rearrange(
                    "p (c r2 xp) -> p c r2 xp", c=C, r2=2
                )
                m1pre_v = M1pre.rearrange("p (r2 cx) -> p r2 cx", r2=2)
                for r2 in range(2):
                    for lo, hi in ((32, 64), (64, 96)):
                        nc.scalar.mul(
                            m1pre_v[lo:hi, r2].rearrange("p (c x) -> p c x", c=C),
                            p1u_c4[lo:hi, :, r2, 64:192],
                            0.25,
                        )
                # single rearrange DMA: [64p][r2:2][384] -> [128p][384]
                # (a partition-split *dst* AP passes the verifier but writes
                # garbage on odd partitions, so the split lives on src side)
                M1 = pool_m1.tile([128, C * 128], f32, name="M1")
                nc.scalar.dma_start(
                    out=M1[:],
                    in_=M1pre[32:96].rearrange("p (r2 cx) -> p r2 cx", r2=2),
                )

                # ---- 4. level-2 map ----
                V4 = pool_v4.tile([128, C * 256], f32, name="V4")
                p1u_c = P1u.rearrange("p (c r2 xp) -> p c r2 xp", c=C, r2=2)
                v4_v = V4.rearrange("p (c xp) -> p c xp", c=C)
                nc.vector.tensor_add(
                    out=v4_v, in0=p1u_c[:, :, 0], in1=p1u_c[:, :, 1]
                )
                M4t = pool_m4t.tile([128, C * 128], f32, name="M4t")
                v4_h = V4.rearrange("p (c X par) -> p c X par", c=C, par=2)
                m4t_v = M4t.rearrange("p (c X) -> p c X", c=C)
                nc.vector.tensor_add(
                    out=m4t_v, in0=v4_h[:, :, :, 0], in1=v4_h[:, :, :, 1]
                )
                M4 = pool_m4.tile([128, C * 128], f32, name="M4")
                nc.scalar.mul(M4[:], M4t[:], 1.0 / 16.0)

                # ---- compacted ring-middle columns ----
                # Mxm free: c*64 + side*32 + j2, rows 32..96 only; the four
                # middle tokens per row ({gx 0,1} then {6,7}) become one
                # contiguous 64-float run -> one gather DMA per row.
                M4m = pool_m4m.tile([128, C * 64], f32, name="M4m")
                M1m = pool_m1m.tile([128, C * 64], f32, name="M1m")
                m4t_c = M4t.rearrange("p (c X) -> p c X", c=C)
                m1_c = M1.rearrange("p (c X) -> p c X", c=C)
                m4m_v = M4m.rearrange("p (c s j) -> p c s j", c=C, s=2)
                m1m_v = M1m.rearrange("p (c s j) -> p c s j", c=C, s=2)
                for side, x0 in ((0, 0), (1, 96)):
                    for lo, hi in ((32, 64), (64, 96)):
                        nc.scalar.mul(
                            m4m_v[lo:hi, :, side],
                            m4t_c[lo:hi, :, x0 : x0 + 32],
                            1.0 / 16.0,
                        )
                        nc.vector.tensor_scalar_mul(
                            m1m_v[lo:hi, :, side],
                            m1_c[lo:hi, :, x0 : x0 + 32],
                            1.0,
                        )

                # ---- 5. assemble U ----
                def u_dst(n, wtok, length=None):
                    """U slice for tokens [n, n+wtok) (single c handled by
                    caller via offset), all channels: [i:16 p][c:3][w*16]."""
                    nh, np_ = divmod(n, 80)
                    pbase = b * 32 + nh * 16
                    return U[pbase : pbase + 16].rearrange(
                        "i (c f) -> i c f", c=C
                    )[:, :, np_ * 16 : np_ * 16 + (length or wtok * 16)]

                # level 0: DRAM -> U directly, one DMA per channel
                # src rows 192+16gy+i, cols 192..320; dst tokens n = 8gy+gx.
                for c in range(C):
                    src = img[b, c, 192:320, 192:320].rearrange(
                        "(gy i) x -> i gy x", gy=8
                    )
                    nh, np_ = 0, 0
                    pbase = b * 32
                    dst = U[pbase : pbase + 16].rearrange(
                        "i (c f) -> i c f", c=C
                    )[:, c, 0 : 64 * 16].rearrange("i (gy f) -> i gy f", gy=8)
                    nc.sync.dma_start(out=dst, in_=src)

                # levels 1, 2: full rows gy 0,1,6,7 (one DMA each) and
                # compacted middles gy 2..5 (one DMA each)
                for n0, m, mm, eng in (
                    (64, M1, M1m, nc.sync),
                    (112, M4, M4m, nc.scalar),
                ):
                    k = 0
                    for gy in (0, 1):
                        src = m[16 * gy : 16 * gy + 16].rearrange(
                            "i (c x) -> i c x", c=C
                        )
                        eng.dma_start(out=u_dst(n0 + k, 8), in_=src)
                        k += 8
                    for gy in (2, 3, 4, 5):
                        src = mm[16 * gy : 16 * gy + 16].rearrange(
                            "i (c x) -> i c x", c=C
                        )
                        eng.dma_start(out=u_dst(n0 + k, 4), in_=src)
                        k += 4
                    for gy in (6, 7):
                        src = m[16 * gy : 16 * gy + 16].rearrange(
                            "i (c x) -> i c x", c=C
                        )
                        eng.dma_start(out=u_dst(n0 + k, 8), in_=src)
                        k += 8

                # ---- 6. output DMAs: per (nh, c) ----
                # SBUF iterates (i, n', j); DRAM matches via transpose.
                for nh in range(2):
                    pbase = b * 32 + nh * 16
                    u_src = U[pbase : pbase + 16].rearrange(
                        "i (c n j) -> i c n j", c=C, n=80
                    )
                    for c in range(C):
                        eng = nc.gpsimd if c == 0 else nc.scalar
                        eng.dma_start(
                            out=outp[b, 80 * nh : 80 * nh + 80, c].transpose(
                                [1, 0, 2]
                            ),
                            in_=u_src[:, c],
                        )

    nc.compile()
    return nc


def _get_module():
    with _lock:
        if "nc" not in _cache:
            _cache["nc"] = _build_module()
        return _cache["nc"]


def kernel(images: np.ndarray) -> np.ndarray:
    from concourse.bass_utils import run_bass_kernel_spmd

    images = np.ascontiguousarray(np.asarray(images, dtype=np.float32))
    assert images.shape == (B_FULL, C, S, S), images.shape

    nc = _get_module()
    in_maps = [
        {"images": images[k * B_SHARD : (k + 1) * B_SHARD]} for k in range(N_CORES)
    ]
    res = run_bass_kernel_spmd(
        nc,
        in_maps,
        core_ids=list(range(N_CORES)),
        trace=bool(int(os.environ.get("FOV_TRACE", "0"))),
    )
    _cache["last_results"] = res
    out = np.concatenate([r["out"] for r in res.results], axis=0)
    return out


if __name__ == "__main__":
    x = np.random.randn(B_FULL, C, S, S).astype(np.float32)
    y = kernel(x)
    print("out", y.shape, y.dtype, float(np.abs(y).max()))


# revision 14
# speedup vs baseline: 1.4106x; 1.1153x over previous
"""BatchedFoveator Trainium2 kernel.

The reference computes an integral image (double cumsum) and gathers 4
corners per output pixel.  Mathematically that is exactly multi-scale
average pooling of the input image:

  level 0 (stride 1, 64 tokens): center crop  rows/cols [192, 320)
  level 1 (stride 2, 48 tokens): 2x2 average pool of  [128, 384)^2 (ring)
  level 2 (stride 4, 48 tokens): 4x4 average pool of full image   (ring)

Each level yields a 128x128 map per (b, c); token (gy, gx) of a level is
the 16x16 tile map[16gy:16gy+16, 16gx:16gx+16].  Ring order for levels
1/2: rows gy=0,1 (all gx), then gy=2..5 with gx in {0,1,6,7}, then
gy=6,7 (all gx).

Sharding: pure data parallel, batch 32 -> 4 images per core x 8 cores.

Per-core pipeline (per image b of its 4):
  1. One 3MB HWDGE load -> It [p = rows 4p..4p+4][c*2048 + r*512 + x].
  2. DVE adds: vertical pairs -> P1v, horizontal pairs -> P1u (2x2 box
     sums, two pooled rows per partition).
  3. Level-1 map: ACT-scale center of P1u -> M1pre (r2-major), one
     SBUF->SBUF DMA -> M1 [p = map row][c*128 + X]; ACT compacts the
     ring-middle columns -> M1m.
  4. Level-2 map: DVE pool P1u again -> M4t, ACT scale -> M4 + M4m.
  5. Gather DMAs (SBUF->SBUF) assemble U [p = b*32 + nh*16 + i]
     [c*1280 + n'*16 + j]  (nh = n//80, n' = n%80); level-0 tokens are
     DMA'd DRAM->U straight from the input image crop.
  6. Out DMAs per (b, nh, c): U -> out[b, 80nh:80nh+80, c].

DMA instruction count is the main cost driver (HWDGE issue is ~0.6us
per dma_start), so transfers are batched to the 3-dim AP limit and
spread across the sync/scalar HWDGE rings and the gpsimd SWDGE path.
"""

import os
import threading

import numpy as np

N_CORES = 8
B_FULL = 32
B_SHARD = B_FULL // N_CORES  # 4
C = 3
S = 512
T = 16

_lock = threading.Lock()
_cache = {}


def _build_module():
    import concourse.bacc as bacc
    import concourse.mybir as mybir
    import concourse.tile as tile

    nc = bacc.Bacc("TRN2", target_bir_lowering=False, debug=False)
    f32 = mybir.dt.float32

    images = nc.dram_tensor("images", (B_SHARD, C, S, S), f32, kind="ExternalInput")
    out = nc.dram_tensor("out", (B_SHARD, 160, C, T, T), f32, kind="ExternalOutput")

    img = images.ap()
    outp = out.ap()

    with tile.TileContext(nc) as tc:
        with (
            tc.tile_pool(name="img", bufs=4) as pool_img,
            tc.tile_pool(name="p1v", bufs=2) as pool_p1v,
            tc.tile_pool(name="p1u", bufs=2) as pool_p1u,
            tc.tile_pool(name="m1pre", bufs=2) as pool_m1pre,
            tc.tile_pool(name="m1", bufs=2) as pool_m1,
            tc.tile_pool(name="m1m", bufs=2) as pool_m1m,
            tc.tile_pool(name="v4", bufs=2) as pool_v4,
            tc.tile_pool(name="m4t", bufs=2) as pool_m4t,
            tc.tile_pool(name="m4", bufs=2) as pool_m4,
            tc.tile_pool(name="m4m", bufs=2) as pool_m4m,
            tc.tile_pool(name="uout", bufs=1) as pool_u,
        ):
            # U holds the fully assembled output for all 4 images:
            # partition p = b*32 + nh*16 + i, free = c*1280 + n'*16 + j.
            U = pool_u.tile([128, C * 80 * T], f32, name="U")

            # ---- 1. all input-only DMAs first, so none queues behind a
            # dependent transfer in the HWDGE ring FIFOs ----
            its = []
            for b in range(B_SHARD):
                # It free: c*2048 + r*512 + x  (partition p = rows 4p..4p+4,
                # r = row%4 = 2*r2 + e); 8KB DRAM runs.
                It = pool_img.tile([128, C * 2048], f32, name="It", tag="It")
                img_b = img[b].rearrange("c (p r) x -> p c (r x)", p=128)
                nc.sync.dma_start(out=It[:], in_=img_b)
                its.append(It)

                # level 0: DRAM -> U directly, one DMA per channel
                # src rows 192+16gy+i, cols 192..320; dst tokens n = 8gy+gx.
                pbase = b * 32
                for c in range(C):
                    src = img[b, c, 192:320, 192:320].rearrange(
                        "(gy i) x -> i gy x", gy=8
                    )
                    dst = U[pbase : pbase + 16].rearrange(
                        "i (c f) -> i c f", c=C
                    )[:, c, 0 : 64 * 16].rearrange("i (gy f) -> i gy f", gy=8)
                    nc.scalar.dma_start(out=dst, in_=src)

            for b in range(B_SHARD):
                It = its[b]
                # ---- 2. 2x2 box sums ----
                # P1v free: cr*512 + x  (cr = 2c + r2); partition p holds
                # pooled rows 2p (r2=0), 2p+1 (r2=1).
                P1v = pool_p1v.tile([128, C * 1024], f32, name="P1v")
                it_v = It.rearrange("p (cr e x) -> p cr e x", cr=2 * C, e=2)
                p1v_v3 = P1v.rearrange("p (cr x) -> p cr x", cr=2 * C)
                nc.vector.tensor_add(
                    out=p1v_v3, in0=it_v[:, :, 0], in1=it_v[:, :, 1]
                )
                # P1u free: cr*256 + xp
                P1u = pool_p1u.tile([128, C * 512], f32, name="P1u")
                p1v_h = P1v.rearrange("p (cr xp par) -> p cr xp par", cr=2 * C, par=2)
                p1u_v = P1u.rearrange("p (cr xp) -> p cr xp", cr=2 * C)
                nc.vector.tensor_add(
                    out=p1u_v, in0=p1v_h[:, :, :, 0], in1=p1v_h[:, :, :, 1]
                )

                # ---- 3. level-1 map ----
                # M1pre free: r2*384 + c*128 + X (r2-major so the rearrange
                # to M1 is a single DMA); partitions 32..96.
                # Engine APs starting at partition 32 may span <=32
                # partitions -> split ops per partition half.
                # P1u crop: cols xp 64..192 of cr = 2c+r2.
                M1pre = pool_m1pre.tile([128, 2 * C * 128], f32, name="M1pre")
                p1u_c4 = P1u.rearrange(
                    "p (c r2 xp) -> p c r2 xp", c=C, r2=2
                )
                m1pre_v = M1pre.rearrange("p (r2 cx) -> p r2 cx", r2=2)
                for r2 in range(2):
                    for lo, hi in ((32, 64), (64, 96)):
                        nc.scalar.mul(
                            m1pre_v[lo:hi, r2].rearrange("p (c x) -> p c x", c=C),
                            p1u_c4[lo:hi, :, r2, 64:192],
                            0.25,
                        )
                # single rearrange DMA: [64p][r2:2][384] -> [128p][384]
                # (a partition-split *dst* AP passes the verifier but writes
                # garbage on odd partitions, so the split lives on src side)
                M1 = pool_m1.tile([128, C * 128], f32, name="M1")
                nc.scalar.dma_start(
                    out=M1[:],
                    in_=M1pre[32:96].rearrange("p (r2 cx) -> p r2 cx", r2=2),
                )

                # ---- 4. level-2 map ----
                V4 = pool_v4.tile([128, C * 256], f32, name="V4")
                p1u_c = P1u.rearrange("p (c r2 xp) -> p c r2 xp", c=C, r2=2)
                v4_v = V4.rearrange("p (c xp) -> p c xp", c=C)
                nc.vector.tensor_add(
                    out=v4_v, in0=p1u_c[:, :, 0], in1=p1u_c[:, :, 1]
                )
                M4t = pool_m4t.tile([128, C * 128], f32, name="M4t")
                v4_h = V4.rearrange("p (c X par) -> p c X par", c=C, par=2)
                m4t_v = M4t.rearrange("p (c X) -> p c X", c=C)
                nc.vector.tensor_add(
                    out=m4t_v, in0=v4_h[:, :, :, 0], in1=v4_h[:, :, :, 1]
                )
                M4 = pool_m4.tile([128, C * 128], f32, name="M4")
                nc.scalar.mul(M4[:], M4t[:], 1.0 / 16.0)

                # ---- compacted ring-middle columns ----
                # Mxm free: c*64 + side*32 + j2, rows 32..96 only; the four
                # middle tokens per row ({gx 0,1} then {6,7}) become one
                # contiguous 64-float run -> one gather DMA per row.
                M4m = pool_m4m.tile([128, C * 64], f32, name="M4m")
                M1m = pool_m1m.tile([128, C * 64], f32, name="M1m")
                m4t_c = M4t.rearrange("p (c X) -> p c X", c=C)
                m1_c = M1.rearrange("p (c X) -> p c X", c=C)
                m4m_v = M4m.rearrange("p (c s j) -> p c s j", c=C, s=2)
                m1m_v = M1m.rearrange("p (c s j) -> p c s j", c=C, s=2)
                for side, x0 in ((0, 0), (1, 96)):
                    for lo, hi in ((32, 64), (64, 96)):
                        nc.scalar.mul(
                            m4m_v[lo:hi, :, side],
                            m4t_c[lo:hi, :, x0 : x0 + 32],
                            1.0 / 16.0,
                        )
                        nc.vector.tensor_scalar_mul(
                            m1m_v[lo:hi, :, side],
                            m1_c[lo:hi, :, x0 : x0 + 32],
                            1.0,
                        )

                # ---- 5. assemble U ----
                def u_dst(n, wtok):
                    """U slice for tokens [n, n+wtok), all channels:
                    [i:16 p][c:3][w*16]."""
                    nh, np_ = divmod(n, 80)
                    pbase = b * 32 + nh * 16
                    return U[pbase : pbase + 16].rearrange(
                        "i (c f) -> i c f", c=C
                    )[:, :, np_ * 16 : np_ * 16 + wtok * 16]

                # levels 1, 2: full rows gy 0,1,6,7 (one DMA each) and
                # compacted middles gy 2..5 (one DMA each); spread across
                # the two HWDGE rings
                for n0, m, mm, eng_full, eng_mid in (
                    (64, M1, M1m, nc.sync, nc.scalar),
                    (112, M4, M4m, nc.scalar, nc.sync),
                ):
                    k = 0
                    for gy in (0, 1):
                        src = m[16 * gy : 16 * gy + 16].rearrange(
                            "i (c x) -> i c x", c=C
                        )
                        eng_full.dma_start(out=u_dst(n0 + k, 8), in_=src)
                        k += 8
                    for gy in (2, 3, 4, 5):
                        src = mm[16 * gy : 16 * gy + 16].rearrange(
                            "i (c x) -> i c x", c=C
                        )
                        eng_mid.dma_start(out=u_dst(n0 + k, 4), in_=src)
                        k += 4
                    for gy in (6, 7):
                        src = m[16 * gy : 16 * gy + 16].rearrange(
                            "i (c x) -> i c x", c=C
                        )
                        eng_full.dma_start(out=u_dst(n0 + k, 8), in_=src)
                        k += 8

                # ---- 6. output DMAs: per (nh, c), all on the SWDGE path
                # (HWDGE descriptor-gen for these 1280-descriptor DMAs
                # stalls the ring ~7us each; the Q7 CounterMachine emits
                # descriptors 16 lanes at a time) ----
                for nh in range(2):
                    pbase = b * 32 + nh * 16
                    u_src = U[pbase : pbase + 16].rearrange(
                        "i (c n j) -> i c n j", c=C, n=80
                    )
                    for c in range(C):
                        nc.gpsimd.dma_start(
                            out=outp[b, 80 * nh : 80 * nh + 80, c].transpose(
                                [1, 0, 2]
                            ),
                            in_=u_src[:, c],
                        )

    nc.compile()
    return nc


def _get_module():
    with _lock:
        if "nc" not in _cache:
            _cache["nc"] = _build_module()
        return _cache["nc"]


def kernel(images: np.ndarray) -> np.ndarray:
    from concourse.bass_utils import run_bass_kernel_spmd

    images = np.ascontiguousarray(np.asarray(images, dtype=np.float32))
    assert images.shape == (B_FULL, C, S, S), images.shape

    nc = _get_module()
    in_maps = [
        {"images": images[k * B_SHARD : (k + 1) * B_SHARD]} for k in range(N_CORES)
    ]
    res = run_bass_kernel_spmd(
        nc,
        in_maps,
        core_ids=list(range(N_CORES)),
        trace=bool(int(os.environ.get("FOV_TRACE", "0"))),
    )
    _cache["last_results"] = res
    out = np.concatenate([r["out"] for r in res.results], axis=0)
    return out


if __name__ == "__main__":
    x = np.random.randn(B_FULL, C, S, S).astype(np.float32)
    y = kernel(x)
    print("out", y.shape, y.dtype, float(np.abs(y).max()))
